# revision 2
# baseline (speedup 1.0000x reference)
"""Top-8-per-row kernel for x[2048, 32768] fp32 on 8 TRN2 NeuronCores.

Data-parallel over rows: 256 rows/core = 2 partition blocks of 128.
Raw-semaphore Bass program (no TileContext): the Sync engine's HWDGE
ring streams column tiles into SBUF at the per-core DMA line rate
(~421 GB/s aggregate over the 16 SDMA engines; measured to be a hard
cap — each engine serves descriptors at ~26.4 B/ns read-side, and
neither a second HWDGE ring, SWDGE, bigger descriptors, nor casting
DMAs lift it). DVE MAX8 reduces each tile to its top-8 per partition,
a final MAX8 folds the per-tile candidates, a reversed DVE copy makes
the order ascending, and the Scalar ring stores the result.

Tile sizing balances stream rate against tail latency: the tail after
the last DMA byte is dominated by the last tile's MAX8 (DVE runs at
~1.08 ns/col + ~160 ns/instr, marginally faster than the ~1.19 ns/col
DMA cadence, so it has no slack to pre-drain). Block 0 uses 4096-col
tiles (16KB lines, full line rate); block 1 uses 2048-col tiles (8KB
lines cost ~1% rate on that half) so the exposed tail MAX8 is ~2.3 us
instead of ~4.4 us. Measured: ~95.5 us vs ~96.9 us for the uniform
4096-col TileContext version (fast device state).
"""

from contextlib import ExitStack

import numpy as np

import concourse.bass as bass
from concourse import bacc, mybir
from concourse.bass_utils import run_bass_kernel_spmd

B = 2048
N = 32768
K = 8
N_CORES = 8
ROWS_PER_CORE = B // N_CORES  # 256
P = 128
N_BLOCKS = ROWS_PER_CORE // P  # 2
F32 = mybir.dt.float32

TAPER = [4096] * 8  # block 0: full 16KB-line rate
LAST_TAPER = [2048] * 16  # block 1: short tail MAX8
BUFS = 8


def _build(taper=None, last_taper=None, bufs: int = BUFS) -> bass.Bass:
    taper = list(taper or TAPER)
    tapers = [taper] * (N_BLOCKS - 1) + [list(last_taper or LAST_TAPER)]
    assert all(sum(tp) == N for tp in tapers)
    max_c = max(max(tp) for tp in tapers)

    nc = bacc.Bacc(
        "TRN2", target_bir_lowering=False, debug=False, num_devices=N_CORES
    )
    x = nc.dram_tensor("x", [ROWS_PER_CORE, N], F32, kind="ExternalInput").ap()
    out = nc.dram_tensor("out", [ROWS_PER_CORE, K], F32, kind="ExternalOutput").ap()

    tiles = [(b, t) for b in range(N_BLOCKS) for t in range(len(tapers[b]))]
    n_tiles = len(tiles)
    # Per-slot DMA-completion thresholds: each dma_start bumps its slot
    # sem by 16 (one per SDMA engine); a single shared counting sem
    # would be racy across concurrently-draining transfers.
    slot_used = [0] * bufs
    thresh = []
    for i in range(n_tiles):
        s = i % bufs
        slot_used[s] += 1
        thresh.append(16 * slot_used[s])

    with ExitStack() as ctx:
        block = ctx.enter_context(nc.Block())
        ld = [ctx.enter_context(nc.semaphore(f"ld{s}")) for s in range(bufs)]
        vd = ctx.enter_context(nc.semaphore("vd"))
        fin = ctx.enter_context(nc.semaphore("fin"))
        res = ctx.enter_context(nc.semaphore("res"))
        st = ctx.enter_context(nc.semaphore("st"))
        data = [
            ctx.enter_context(nc.sbuf_tensor(f"data{s}", [P, max_c], F32))
            for s in range(bufs)
        ]
        cands = [
            ctx.enter_context(
                nc.sbuf_tensor(f"cands{b}", [P, K * len(tapers[b])], F32)
            )
            for b in range(N_BLOCKS)
        ]
        top = [
            ctx.enter_context(nc.sbuf_tensor(f"top{b}", [P, K], F32))
            for b in range(N_BLOCKS)
        ]
        asc = [
            ctx.enter_context(nc.sbuf_tensor(f"asc{b}", [P, K], F32))
            for b in range(N_BLOCKS)
        ]

        @block.sync
        def _(eng: bass.BassEngine):
            for i, (b, t) in enumerate(tiles):
                s = i % bufs
                rows = slice(b * P, (b + 1) * P)
                tp = tapers[b]
                off, sz = sum(tp[:t]), tp[t]
                if i >= bufs:
                    eng.wait_ge(vd, i - bufs + 1)
                eng.dma_start(
                    out=data[s][:, :sz], in_=x[rows, off : off + sz]
                ).then_inc(ld[s], 16)

        @block.vector
        def _(vec: bass.BassVectorEngine):
            done = 0
            for i, (b, t) in enumerate(tiles):
                s = i % bufs
                tp = tapers[b]
                sz = tp[t]
                vec.wait_ge(ld[s], thresh[i])
                vec.max(cands[b][:, t * K : (t + 1) * K], data[s][:, :sz]).then_inc(
                    vd, 1
                )
                done += 1
                if t == len(tp) - 1:
                    # DVE writes drain asynchronously: same-engine RAW
                    # needs the sem wait for visibility, not just
                    # program order.
                    vec.wait_ge(vd, done)
                    vec.max(top[b][:], cands[b][:]).then_inc(fin, 1)
                    vec.wait_ge(fin, b + 1)
                    vec.tensor_copy(asc[b][:], top[b][:, ::-1]).then_inc(res, 1)

        @block.scalar
        def _(eng: bass.BassEngine):
            for b in range(N_BLOCKS):
                rows = slice(b * P, (b + 1) * P)
                eng.wait_ge(res, b + 1)
                eng.dma_start(out=out[rows, :], in_=asc[b][:]).then_inc(st, 16)
            eng.wait_ge(st, 16 * N_BLOCKS)

    nc.compile()
    return nc


def kernel(x: np.ndarray, k) -> np.ndarray:
    k = int(np.asarray(k))
    assert k == K, f"kernel hardcoded for k={K}, got {k}"
    assert x.shape == (B, N), x.shape
    x = np.ascontiguousarray(x, dtype=np.float32)

    nc = _build()
    in_maps = [
        {"x": x[c * ROWS_PER_CORE : (c + 1) * ROWS_PER_CORE]} for c in range(N_CORES)
    ]
    res = run_bass_kernel_spmd(nc, in_maps, list(range(N_CORES)))
    out = np.concatenate([res.results[c]["out"] for c in range(N_CORES)], axis=0)
    return np.asarray(out, dtype=np.float32)


if __name__ == "__main__":
    rng = np.random.default_rng(0)
    xs = rng.standard_normal((B, N), dtype=np.float32)
    got = kernel(xs, 8)
    want = np.sort(xs, axis=1)[:, -K:]
    err = np.max(np.abs(got - want))
    print("absmax err:", err)


# revision 6
# speedup vs baseline: 1.1657x; 1.1657x over previous
"""Top-8-per-row kernel for x[2048, 32768] fp32 on 8 TRN2 NeuronCores.

Data-parallel over rows: 256 rows/core = 2 partition blocks of 128.
Raw-semaphore Bass program (no TileContext): the Sync engine's HWDGE
ring streams column tiles into SBUF at the per-core DMA line rate
(~421 GB/s aggregate over the 16 SDMA engines; measured to be a hard
cap — each engine serves descriptors at ~26.4 B/ns read-side, and
neither a second HWDGE ring, SWDGE, bigger descriptors, nor casting
DMAs lift it). DVE MAX8 reduces each tile to its top-8 per partition,
a final MAX8 folds the per-tile candidates, a reversed DVE copy makes
the order ascending, and the Scalar ring stores the result.

Tile sizing balances stream rate against tail latency: the tail after
the last DMA byte is dominated by the last tile's MAX8 (DVE runs at
~1.08 ns/col + ~160 ns/instr, marginally faster than the ~1.19 ns/col
DMA cadence, so it has no slack to pre-drain). Block 0 uses 4096-col
tiles (16KB lines, full line rate); block 1 uses 2048-col tiles (8KB
lines cost ~1% rate on that half) so the exposed tail MAX8 is ~2.3 us
instead of ~4.4 us. The final MAX8 writes straight into a reversed
view of the output staging tile (descending into a reversed AP =
ascending in memory), skipping a sem round-trip and a copy.
Measured: ~95.6 us vs ~96.9 us for the uniform 4096-col TileContext
version (fast device state).
"""

from contextlib import ExitStack

import numpy as np

import concourse.bass as bass
from concourse import bacc, mybir
from concourse.bass_utils import run_bass_kernel_spmd

B = 2048
N = 32768
K = 8
N_CORES = 8
ROWS_PER_CORE = B // N_CORES  # 256
P = 128
N_BLOCKS = ROWS_PER_CORE // P  # 2
F32 = mybir.dt.float32

TAPER = [4096] * 8  # block 0: full 16KB-line rate
LAST_TAPER = [2048] * 16  # block 1: short tail MAX8
BUFS = 8


def _build(taper=None, last_taper=None, bufs: int = BUFS) -> bass.Bass:
    taper = list(taper or TAPER)
    tapers = [taper] * (N_BLOCKS - 1) + [list(last_taper or LAST_TAPER)]
    assert all(sum(tp) == N for tp in tapers)
    max_c = max(max(tp) for tp in tapers)

    nc = bacc.Bacc(
        "TRN2", target_bir_lowering=False, debug=False, num_devices=N_CORES
    )
    x = nc.dram_tensor("x", [ROWS_PER_CORE, N], F32, kind="ExternalInput").ap()
    out = nc.dram_tensor("out", [ROWS_PER_CORE, K], F32, kind="ExternalOutput").ap()

    tiles = [(b, t) for b in range(N_BLOCKS) for t in range(len(tapers[b]))]
    n_tiles = len(tiles)
    # Per-slot DMA-completion thresholds: each dma_start bumps its slot
    # sem by 16 (one per SDMA engine); a single shared counting sem
    # would be racy across concurrently-draining transfers.
    slot_used = [0] * bufs
    thresh = []
    for i in range(n_tiles):
        s = i % bufs
        slot_used[s] += 1
        thresh.append(16 * slot_used[s])

    with ExitStack() as ctx:
        block = ctx.enter_context(nc.Block())
        ld = [ctx.enter_context(nc.semaphore(f"ld{s}")) for s in range(bufs)]
        vd = ctx.enter_context(nc.semaphore("vd"))
        res = ctx.enter_context(nc.semaphore("res"))
        st = ctx.enter_context(nc.semaphore("st"))
        data = [
            ctx.enter_context(nc.sbuf_tensor(f"data{s}", [P, max_c], F32))
            for s in range(bufs)
        ]
        cands = [
            ctx.enter_context(
                nc.sbuf_tensor(f"cands{b}", [P, K * len(tapers[b])], F32)
            )
            for b in range(N_BLOCKS)
        ]
        asc = [
            ctx.enter_context(nc.sbuf_tensor(f"asc{b}", [P, K], F32))
            for b in range(N_BLOCKS)
        ]

        @block.sync
        def _(eng: bass.BassEngine):
            for i, (b, t) in enumerate(tiles):
                s = i % bufs
                rows = slice(b * P, (b + 1) * P)
                tp = tapers[b]
                off, sz = sum(tp[:t]), tp[t]
                if i >= bufs:
                    eng.wait_ge(vd, i - bufs + 1)
                eng.dma_start(
                    out=data[s][:, :sz], in_=x[rows, off : off + sz]
                ).then_inc(ld[s], 16)

        @block.vector
        def _(vec: bass.BassVectorEngine):
            done = 0
            for i, (b, t) in enumerate(tiles):
                s = i % bufs
                tp = tapers[b]
                sz = tp[t]
                vec.wait_ge(ld[s], thresh[i])
                vec.max(cands[b][:, t * K : (t + 1) * K], data[s][:, :sz]).then_inc(
                    vd, 1
                )
                done += 1
                if t == len(tp) - 1:
                    # DVE writes drain asynchronously: same-engine RAW
                    # needs the sem wait for visibility, not just
                    # program order.
                    vec.wait_ge(vd, done)
                    # Descending MAX8 into a reversed view = ascending
                    # in memory; skips a sem round-trip and a copy.
                    vec.max(asc[b][:, ::-1], cands[b][:]).then_inc(res, 1)

        @block.scalar
        def _(eng: bass.BassEngine):
            for b in range(N_BLOCKS):
                rows = slice(b * P, (b + 1) * P)
                eng.wait_ge(res, b + 1)
                eng.dma_start(out=out[rows, :], in_=asc[b][:]).then_inc(st, 16)
            eng.wait_ge(st, 16 * N_BLOCKS)

    nc.compile()
    return nc


def kernel(x: np.ndarray, k) -> np.ndarray:
    k = int(np.asarray(k))
    assert k == K, f"kernel hardcoded for k={K}, got {k}"
    assert x.shape == (B, N), x.shape
    x = np.ascontiguousarray(x, dtype=np.float32)

    nc = _build()
    in_maps = [
        {"x": x[c * ROWS_PER_CORE : (c + 1) * ROWS_PER_CORE]} for c in range(N_CORES)
    ]
    res = run_bass_kernel_spmd(nc, in_maps, list(range(N_CORES)))
    out = np.concatenate([res.results[c]["out"] for c in range(N_CORES)], axis=0)
    return np.asarray(out, dtype=np.float32)


if __name__ == "__main__":
    rng = np.random.default_rng(0)
    xs = rng.standard_normal((B, N), dtype=np.float32)
    got = kernel(xs, 8)
    want = np.sort(xs, axis=1)[:, -K:]
    err = np.max(np.abs(got - want))
    print("absmax err:", err)
